# revision 21
# baseline (speedup 1.0000x reference)
"""Trainium2 Bass kernel for AttentionBilinear.

Per batch b:
    pW     = p[b] @ W                         # [Tp, Dq]
    scores = pW @ q[b].T                      # [Tp, Tq]
    wts    = softmax(scores, axis=Tp)
    out[b] = wts @ q[b]                       # [Tp, Dq]

Computed in the transposed-scores orientation so the softmax over Tp is a
free-axis reduction:
    pWT[d, tp]  = sum_e W[e, d] * pT[e, tp]       (mm1: lhsT=W,  rhs=pT)
    scT[tq, tp] = sum_d qT[d, tq] * pWT[d, tp]    (mm2: lhsT=qT, rhs=pWT)
    softmax over tp (free axis), read from PSUM   (DVE max / ACT exp / DVE mul)
    out[tp, d]  = sum_tq wT[tq, tp] * q[tq, d]    (mm3: lhsT=wT, rhs=q)

fp16 matmul runs at 1 cycle/row (fp32 is 4); all three GEMMs need ~fp16
accuracy (the harness metric is max-abs-err / absmax, 2e-2), so the kernel
is PE-bound at ~164us/core. Everything else is startup/tail engineering:

  - W is host-blocked k-major ([ce, p, m, c]) and pT k-major ([b, k, n, p, h])
    so batch 0's first mm1 half runs k-outer: the PE starts after ONE 256KB
    W piece + ONE 128KB pT chunk instead of a 1MB+ halves. Chunked DMAs
    stream in consumption order on the sync ring while W rides scalar.
  - The PE warm-up junk matmuls bridge the ~7us engine preamble so the HAM
    clock-gate is released (and stays released) when real work starts.
  - mm1's k-outer bulk drains alternate ACT/DVE so the n=1 pass never waits
    on PSUM banks.
  - Output is stored fp16 (upcast on host): halves the store tail; stores
    ride the otherwise-idle gpsimd (SWDGE) queue so the sync ring only
    carries loads.

Sharding: data-parallel over batch B=16 across 8 cores, W replicated.
"""

import numpy as np

P = 128   # partitions
H = 512   # PSUM bank width in fp32

B_FULL = 16
T_FULL = 1024
D_FULL = 1024
N_CORES = 8

MODE = ("hi", "hi")  # kept for the test harness printout

N_WARM = 7


def build_nc(b_loc=2, t=1024, d=1024, mode=MODE):
    from contextlib import ExitStack

    import concourse.tile as tile
    from concourse import bacc, mybir

    f32 = mybir.dt.float32
    f16 = mybir.dt.float16
    C = t // P     # row chunks of a [t, d] matrix
    KC = d // P    # chunks of the d (feature) axis
    TH = t // H    # 512-wide pieces of the t axis
    NH = d // H    # 512-wide pieces of the d axis
    AX = mybir.AxisListType.X
    EXP = mybir.ActivationFunctionType.Exp
    MIN = mybir.AluOpType.min
    ADD = mybir.AluOpType.add

    nc = bacc.Bacc()

    # Inputs are host-packed partition-major so every DMA descriptor is a
    # >=1KB contiguous run.
    def dram_in(name):
        return nc.dram_tensor(
            name, [b_loc, P, C, d], f16, kind="ExternalInput"
        ).ap()

    qh_ext = dram_in("qh")          # q natural, fp16: [b, p, c, d]
    qt_ext = dram_in("qt")          # q transposed per batch: [b, p, c, tq]
    # p transposed, k-major chunks: [b, k, n, p, H] = pT[e=k*128+p, tp=n*512+h]
    pt_ext = nc.dram_tensor(
        "pt", [b_loc, KC, TH, P, H], f16, kind="ExternalInput"
    ).ap()
    # W blocked k-major: [ce, p, m, c] = W[ce*128+p, m*128+c]; each ce-piece
    # is one contiguous 256KB DMA and is the unit mm1's k-loop consumes.
    w_ext = nc.dram_tensor("w", [KC, P, KC, P], f16, kind="ExternalInput").ap()
    out_ext = nc.dram_tensor("out", [b_loc, t, d], f16, kind="ExternalOutput").ap()

    with tile.TileContext(nc) as tc, ExitStack() as ctx:
        consts = ctx.enter_context(tc.tile_pool(name="consts", bufs=1))
        qh_pool = ctx.enter_context(tc.tile_pool(name="qh_pool", bufs=2))
        qt_pool = ctx.enter_context(tc.tile_pool(name="qt_pool", bufs=2))
        pt_pool = ctx.enter_context(tc.tile_pool(name="pt_pool", bufs=2))
        pwt_pool = ctx.enter_context(tc.tile_pool(name="pwt_pool", bufs=2))
        wt_pool = ctx.enter_context(tc.tile_pool(name="wt_pool", bufs=2))
        ostage = ctx.enter_context(tc.tile_pool(name="ostage", bufs=4))
        stats = ctx.enter_context(tc.tile_pool(name="stats", bufs=2))
        psum_mm = ctx.enter_context(tc.tile_pool(name="psum_mm", bufs=8, space="PSUM"))

        # ---- PE warm-up: junk matmuls bridge the engine preamble (~7us)
        # until the first data lands, releasing the HAM clock-gate ----
        warm = consts.tile([P, H], f16, name="warm")
        nc.gpsimd.memset(warm[:], 0.0)
        wacc = psum_mm.tile([P, H], f32, name="wacc", tag="acc")
        for i in range(N_WARM):
            nc.tensor.matmul(
                wacc[:], warm[:, 0:P], warm[:], start=(i == 0), stop=(i == N_WARM - 1)
            )
        # NOTE: N=128 warm-up matmuls (more, smaller) measured 218us vs 183us
        # for this N=512 form — small junk MMs keep the HAM clock-gate
        # oscillating. Keep N=512.

        # ---- W resident (fp16), [p, ce(k), m, c]; lhsT for (k, m) is
        # w[:, k, m]. Loaded on the scalar ring (idle at startup): the two
        # first-consumed k-pieces individually, the rest as one DMA. ----
        w_sb = consts.tile([P, KC, KC, P], f16, name="w_hi")
        # One trigger per k-piece: trigger-issue pace (~0.7us each) is what
        # bounds how soon piece k can land, so no splitting.
        for k in range(KC):
            nc.scalar.dma_start(w_sb[:, k], w_ext[k])

        st = [dict() for _ in range(b_loc)]

        def load_mat(pool, name, tag, ext, b, engine):
            """packed [b, P, C, d] DRAM (fp16) -> [P, C, d] SBUF in one DMA."""
            mt = pool.tile([P, C, d], f16, name=name, tag=tag)
            engine.dma_start(mt[:], ext[b])
            return mt

        def phase_loads(b):
            # All loads ride the sync ring as one FIFO in exact consumption
            # order, so early phases are never starved by later tensors.
            pt_t = pt_pool.tile([P, KC, TH, H], f16, name=f"pT_{b}", tag="pT")
            # Per-chunk DMAs in mm1's consumption order (n outer for b>0,
            # k-outer n=0 first for b=0); batch 0's first matmul only needs
            # chunk (0, 0). Source slices match dst AP order exactly.
            for n in range(TH):
                for k in range(KC):
                    nc.sync.dma_start(pt_t[:, k, n], pt_ext[b, k, n])
            st[b]["pT"] = pt_t
            st[b]["qT"] = load_mat(qt_pool, f"qT_{b}", "qT", qt_ext, b, nc.sync)
            st[b]["qh"] = load_mat(qh_pool, f"qh_{b}", "qh", qh_ext, b, nc.sync)

        def phase_mm1(b):
            """pWT[d, tp] = sum_e W[e,d] * pT[e,tp]."""
            pT = st[b]["pT"]
            pWT = pwt_pool.tile([P, KC, t], f16, name=f"pWT_{b}", tag="pWT")
            if b == 0:
                # n=0 half k-outer: consumes (w piece k, pT chunk k) pairs as
                # they stream in; 8 accs = all 8 PSUM banks.
                accs = [
                    psum_mm.tile([P, H], f32, name=f"a1_0_{m}_0", tag="acc")
                    for m in range(KC)
                ]
                for k in range(KC):
                    for m in range(KC):
                        nc.tensor.matmul(
                            accs[m][:],
                            w_sb[:, k, m],
                            pT[:, k, 0],
                            start=(k == 0),
                            stop=(k == KC - 1),
                        )
                # Bulk drains alternate ACT/DVE so the n=1 pass never waits
                # on a PSUM bank.
                for m, acc in enumerate(accs):
                    if m % 2 == 0:
                        nc.scalar.copy(pWT[:, m, 0:H], acc[:])
                    else:
                        nc.vector.tensor_copy(pWT[:, m, 0:H], acc[:])
                n_range = range(1, TH)
            else:
                n_range = range(TH)
            for n in n_range:
                for m in range(KC):
                    acc = psum_mm.tile([P, H], f32, name=f"a1_{b}_{m}_{n}", tag="acc")
                    for k in range(KC):
                        nc.tensor.matmul(
                            acc[:],
                            w_sb[:, k, m],
                            pT[:, k, n],
                            start=(k == 0),
                            stop=(k == KC - 1),
                        )
                    if m % 2 == 0:
                        nc.scalar.copy(pWT[:, m, n * H : (n + 1) * H], acc[:])
                    else:
                        nc.vector.tensor_copy(pWT[:, m, n * H : (n + 1) * H], acc[:])
            st[b]["pWT"] = pWT

        def phase_mm2sm(b):
            """scores into PSUM; softmax straight out of PSUM into fp16 wT."""
            qT = st[b]["qT"]
            pWT = st[b]["pWT"]
            wT = wt_pool.tile([P, C, t], f16, name=f"wT_{b}", tag="wT")
            negmax = stats.tile([P, C, TH], f32, name=f"negmax_{b}", tag="negmax")
            nm = stats.tile([P, C], f32, name=f"nm_{b}", tag="nm")
            sume = stats.tile([P, C, TH], f32, name=f"sume_{b}", tag="sume")
            recip = stats.tile([P, C], f32, name=f"recip_{b}", tag="recip")
            for m in range(C):
                msl = slice(m * P, (m + 1) * P)
                accs = []
                for n in range(TH):
                    acc = psum_mm.tile([P, H], f32, name=f"a2_{b}_{m}_{n}", tag="acc")
                    for k in range(KC):
                        nc.tensor.matmul(
                            acc[:],
                            qT[:, k, msl],
                            pWT[:, k, n * H : (n + 1) * H],
                            start=(k == 0),
                            stop=(k == KC - 1),
                        )
                    nc.vector.reduce_max(
                        negmax[:, m, n : n + 1], acc[:], axis=AX, negate=True
                    )
                    accs.append(acc)
                if TH > 1:
                    nc.vector.tensor_tensor(
                        nm[:, m : m + 1], negmax[:, m, 0:1], negmax[:, m, 1:2], op=MIN
                    )
                    nm_sl = nm[:, m : m + 1]
                else:
                    nm_sl = negmax[:, m, 0:1]
                for n, acc in enumerate(accs):
                    nc.scalar.activation(
                        wT[:, m, n * H : (n + 1) * H],
                        acc[:],
                        EXP,
                        bias=nm_sl,
                        accum_out=sume[:, m, n : n + 1],
                    )
                if TH > 1:
                    nc.vector.tensor_tensor(
                        recip[:, m : m + 1], sume[:, m, 0:1], sume[:, m, 1:2], op=ADD
                    )
                    nc.vector.reciprocal(recip[:, m : m + 1], recip[:, m : m + 1])
                else:
                    nc.vector.reciprocal(recip[:, m : m + 1], sume[:, m, 0:1])
                nc.vector.tensor_scalar_mul(wT[:, m, :], wT[:, m, :], recip[:, m : m + 1])
            st[b]["wT"] = wT

        def phase_mm3(b):
            """out[tp, d] = sum_tq wT[tq,tp] * qh[tq,d]; fp16 drains + stores
            alternate between the ACT/scalar and DVE/sync HWDGE paths so
            consecutive tiles' drain+trigger+receipt chains overlap."""
            wT = st[b]["wT"]
            qh = st[b]["qh"]

            def drain_store(b, m, n, acc):
                ot = ostage.tile([P, H], f16, name=f"ot_{b}_{m}_{n}", tag="ot")
                # Alternate drain/store between (ACT, scalar queue) and
                # (DVE, sync queue) so consecutive tiles' drain + trigger
                # + receipt chains overlap — including the last two.
                if n % 2 == 0:
                    nc.scalar.copy(ot[:], acc[:])
                    nc.scalar.dma_start(
                        out_ext[b, m * P : (m + 1) * P, n * H : (n + 1) * H], ot[:]
                    )
                else:
                    nc.vector.tensor_copy(ot[:], acc[:])
                    nc.sync.dma_start(
                        out_ext[b, m * P : (m + 1) * P, n * H : (n + 1) * H], ot[:]
                    )

            tiles = [(m, n) for m in range(C) for n in range(NH)]
            if b == b_loc - 1:
                # Last batch: no later phase pads the softmax tail, so the
                # k-loop of an early tile would reach chunk k=7 (+1.5us)
                # before m'=7's normalize chain (~3us) completes. Interleave
                # the first 4 tiles' accumulations over k so chunk k is first
                # touched at +4*k*213ns.
                group, tiles = tiles[:4], tiles[4:]
                accs = [
                    psum_mm.tile([P, H], f32, name=f"a3_{b}_g{i}", tag="acc")
                    for i in range(len(group))
                ]
                for k in range(C):
                    for acc, (m, n) in zip(accs, group):
                        nc.tensor.matmul(
                            acc[:],
                            wT[:, k, m * P : (m + 1) * P],
                            qh[:, k, n * H : (n + 1) * H],
                            start=(k == 0),
                            stop=(k == C - 1),
                        )
                for acc, (m, n) in zip(accs, group):
                    drain_store(b, m, n, acc)
            for m, n in tiles:
                msl = slice(m * P, (m + 1) * P)
                n_sl = slice(n * H, (n + 1) * H)
                acc = psum_mm.tile([P, H], f32, name=f"a3_{b}_{m}_{n}", tag="acc")
                for k in range(C):
                    nc.tensor.matmul(
                        acc[:],
                        wT[:, k, msl],
                        qh[:, k, n_sl],
                        start=(k == 0),
                        stop=(k == C - 1),
                    )
                if b == b_loc - 1 and m == C - 1 and n == NH - 1:
                    # The very last tile's drain+store chain is fully exposed
                    # at the kernel end: store it as two parallel half-tiles
                    # on the sync and scalar queues to halve the transfer leg.
                    ot = ostage.tile([P, H], f16, name=f"ot_{b}_{m}_{n}", tag="ot")
                    nc.vector.tensor_copy(ot[:], acc[:])
                    half = H // 2
                    nc.sync.dma_start(
                        out_ext[b, msl, n * H : n * H + half], ot[:, 0:half]
                    )
                    nc.scalar.dma_start(
                        out_ext[b, msl, n * H + half : (n + 1) * H], ot[:, half:H]
                    )
                else:
                    drain_store(b, m, n, acc)

        # Emission order = per-engine program order. Batch b+1's mm1 is
        # emitted before batch b's mm3 so the PE stays busy while b's softmax
        # tail completes.
        phase_loads(0)
        phase_mm1(0)
        for b in range(b_loc):
            phase_mm2sm(b)
            if b + 1 < b_loc:
                phase_loads(b + 1)
                phase_mm1(b + 1)
            phase_mm3(b)

    nc.finalize()  # run the Bacc legalization/regalloc passes for walrus
    return nc


_CACHE = {}


def _get_nc(mode=MODE):
    key = mode
    if key not in _CACHE:
        _CACHE[key] = build_nc(B_FULL // N_CORES, T_FULL, D_FULL, mode=mode)
    return _CACHE[key]


def _prep_inputs(q, p, W, mode=MODE):
    """Host-side layout prep: fp16 casts and per-batch transposes/blocking."""
    q = np.ascontiguousarray(q, dtype=np.float32)
    p = np.ascontiguousarray(p, dtype=np.float32)
    W = np.ascontiguousarray(W, dtype=np.float32)
    d = W.shape[0]
    KC = d // P
    t = q.shape[1]
    C = t // P
    TH = t // H

    def pack(x16):
        # [b, t, cols] -> [b, p, c, cols]: 16KB contiguous per partition
        b, _, cols = x16.shape
        return np.ascontiguousarray(
            x16.reshape(b, C, P, cols).transpose(0, 2, 1, 3)
        )

    qh = q.astype(np.float16)
    qt = np.transpose(qh, (0, 2, 1))
    # pT k-major chunks: [b, k, n, p, h] = p[b].T[k*128+p, n*512+h]
    pt = np.transpose(p, (0, 2, 1)).astype(np.float16)
    ptk = np.ascontiguousarray(
        pt.reshape(-1, KC, P, TH, H).transpose(0, 1, 3, 2, 4)
    )
    # W k-major blocks: [ce, p, m, c] = W[ce*128+p, m*128+c]
    wk = np.ascontiguousarray(W.astype(np.float16).reshape(KC, P, KC, P))
    return {"qh": pack(qh), "qt": pack(qt), "pt": ptk, "w": wk}


def run(q, p, W, mode=MODE, nc=None, **spmd_kwargs):
    """Run on 8 NeuronCores; returns (out, BassKernelResults)."""
    from concourse.bass_utils import run_bass_kernel_spmd

    arrs = _prep_inputs(q, p, W, mode=mode)
    if nc is None:
        nc = _get_nc(mode)
    bl = B_FULL // N_CORES
    batch_sharded = {"qh", "qt", "pt"}
    in_maps = []
    for i in range(N_CORES):
        m = {}
        for name, a in arrs.items():
            m[name] = a[i * bl : (i + 1) * bl] if name in batch_sharded else a
        in_maps.append(m)
    res = run_bass_kernel_spmd(nc, in_maps, list(range(N_CORES)), **spmd_kwargs)
    out = np.concatenate(
        [res.results[i]["out"].astype(np.float32) for i in range(N_CORES)], axis=0
    )
    return out, res


def kernel(q, p, W):
    out, _ = run(q, p, W)
    return out
